# revision 2
# baseline (speedup 1.0000x reference)
"""Sparse-attention Trainium2 kernel v2 (nn_Attention_44341242364527).

Head-tensor-parallel over 8 NeuronCores (2 heads/core).  Dense scatter
reformulation of the sparse gather (WT = scatter of exp(geo_bias), causal
mask folded in), as in v1, with these changes:

- Softmax denominator Z is no longer a PE matmul: at-tiles accumulate into
  a per-(j,h) f32 SBUF tile (DVE for h=0, GPSIMD for h=1), then one GPSIMD
  partition_all_reduce gives Z broadcast across partitions; DVE reciprocal
  + multiply normalize.  Removes ~16us of PE streaming (Z + recip-bcast
  matmuls).
- bf16 for at/Vsb/AOT/wo (DVE 2x mode on the exp*wt multiply, FWL on
  matmul weight loads).  QT/KT stay f32r so exp() input error stays tiny.
- Dedicated PSUM pools per stage (proj/ST/AV/Y, 2 banks each) instead of a
  shared pool; psY=1 in v1 serialized the output projection tail.
- Batched DMA: hs chunk is one 2MB transfer (host pre-arranges hsT as
  [P, NK, S]); QKV weights one transfer each; y written per 128-row block.
- Engine balance: V-proj and y copies on ACT, QT/KT copies + multiplies +
  normalize on DVE, Z reduction on GPSIMD.
"""

import math
import sys

sys.path.insert(0, "/opt/trn_rl_repo")

import numpy as np

B, S, H, D, KS = 1, 2048, 16, 128, 64
HID = H * D
NCORES = 8
HPC = H // NCORES          # heads per core
CPC = HPC * D              # output channels per core
P = 128                    # partitions
SC = 512                   # s-chunk (PSUM bank width in f32)
NJ = S // SC               # 4 s-chunks
NT = S // P                # 16 s'-tiles
NK = HID // P              # 16 contraction chunks

_CACHE = {}

CFG = dict(psP=2, psS=2, psA=2, psY=2, lag=3, atp=6, wtp=5)


def _np_bf16():
    import ml_dtypes

    return np.dtype(ml_dtypes.bfloat16)


def _build_nc(reps=1, cfg=None):
    import concourse.tile as tile
    from concourse import bacc, bass_isa, mybir

    c = dict(CFG)
    if cfg:
        c.update(cfg)

    F32 = mybir.dt.float32
    F32R = mybir.dt.float32r
    BF16 = mybir.dt.bfloat16
    EXP = mybir.ActivationFunctionType.Exp
    MULT = mybir.AluOpType.mult
    ADD = mybir.AluOpType.add
    RADD = bass_isa.ReduceOp.add

    nc = bacc.Bacc("TRN2", target_bir_lowering=False, debug=False,
                   num_devices=NCORES)

    hsr = nc.dram_tensor("hsr", [P, NK, S], BF16, kind="ExternalInput")
    wqr = nc.dram_tensor("wqr", [P, NK, CPC], BF16, kind="ExternalInput")
    wkr = nc.dram_tensor("wkr", [P, NK, CPC], BF16, kind="ExternalInput")
    wvr = nc.dram_tensor("wvr", [P, NK, CPC], BF16, kind="ExternalInput")
    wor = nc.dram_tensor("wor", [CPC, HID], BF16, kind="ExternalInput")
    wt = nc.dram_tensor("wt", [HPC, S, S], BF16, kind="ExternalInput")
    y = nc.dram_tensor("y", [S, HID], BF16, kind="ExternalOutput")

    inv_sqrt_d = 1.0 / math.sqrt(D)

    def mm(out, lhsT, rhs, **kw):
        nc.tensor.matmul(out, lhsT, rhs, **kw)

    with tile.TileContext(nc) as tc, \
            nc.allow_low_precision(reason="bf16/f32r matmul operands; PSUM f32"):
        with tc.tile_pool(name="persist", bufs=1) as persist:
            QT = [persist.tile([P, S], F32R, tag=f"qt{h}", name=f"qt{h}")
                  for h in range(HPC)]
            KT = [persist.tile([P, S], F32R, tag=f"kt{h}", name=f"kt{h}")
                  for h in range(HPC)]
            Vsb = [persist.tile([P, CPC], BF16, tag=f"v{t}", name=f"vres{t}")
                   for t in range(NT)]
            AOT = [persist.tile([P, S], BF16, tag=f"aot{h}", name=f"aot{h}")
                   for h in range(HPC)]

            with tc.tile_pool(name="wpool", bufs=1) as wpool, \
                 tc.tile_pool(name="hpool", bufs=2) as hpool, \
                 tc.tile_pool(name="wtp", bufs=c["wtp"]) as wtp, \
                 tc.tile_pool(name="atp", bufs=c["atp"]) as atp, \
                 tc.tile_pool(name="zpool", bufs=4) as zpool, \
                 tc.tile_pool(name="zbp", bufs=2) as zbp, \
                 tc.tile_pool(name="rbp", bufs=2) as rbp, \
                 tc.tile_pool(name="ypool", bufs=2) as ypool, \
                 tc.tile_pool(name="psP", bufs=c["psP"], space="PSUM") as psP, \
                 tc.tile_pool(name="psS", bufs=c["psS"], space="PSUM") as psS, \
                 tc.tile_pool(name="psA", bufs=c["psA"], space="PSUM") as psA, \
                 tc.tile_pool(name="psY", bufs=c["psY"], space="PSUM") as psY:
                wq_sb = wk_sb = wv_sb = None
                wo_sb = []

                for _rep in range(reps):
                    for j in range(NJ):
                        # -- batched input DMA for chunk j --
                        hs_j = hpool.tile([P, NK * SC], BF16, tag="hs",
                                          name="hs")
                        nc.sync.dma_start(
                            hs_j[:], hsr[:, :, j * SC:(j + 1) * SC])
                        if _rep == 0 and j == 0:
                            wq_sb = wpool.tile([P, NK * CPC], BF16, tag="wq",
                                               name="wq")
                            wk_sb = wpool.tile([P, NK * CPC], BF16, tag="wk",
                                               name="wk")
                            wv_sb = wpool.tile([P, NK * CPC], BF16, tag="wv",
                                               name="wv")
                            nc.sync.dma_start(wq_sb[:], wqr[:, :, :])
                            nc.sync.dma_start(wk_sb[:], wkr[:, :, :])
                            nc.sync.dma_start(wv_sb[:], wvr[:, :, :])
                            for h in range(HPC):
                                t_ = wpool.tile([P, HID], BF16, tag=f"wo{h}",
                                                name=f"wo{h}")
                                nc.sync.dma_start(
                                    t_[:], wor[h * P:(h + 1) * P, :])
                                wo_sb.append(t_)

                        # -- QKV projection for chunk j --
                        for h in range(HPC):
                            for w_sb, acc in ((wq_sb, QT), (wk_sb, KT)):
                                pp = psP.tile([P, SC], F32, tag="p",
                                              name="ps_proj")
                                for k in range(NK):
                                    mm(pp[:],
                                       w_sb[:, k * CPC + h * D:
                                            k * CPC + (h + 1) * D],
                                       hs_j[:, k * SC:(k + 1) * SC],
                                       start=(k == 0), stop=(k == NK - 1))
                                nc.vector.tensor_copy(
                                    acc[h][:, j * SC:(j + 1) * SC], pp[:])
                        for si in range(SC // P):
                            vp = psP.tile([P, CPC], F32, tag="p",
                                          name="ps_projv")
                            for k in range(NK):
                                mm(vp[:],
                                   hs_j[:, k * SC + si * P:
                                        k * SC + (si + 1) * P],
                                   wv_sb[:, k * CPC:(k + 1) * CPC],
                                   start=(k == 0), stop=(k == NK - 1))
                            nc.scalar.copy(Vsb[4 * j + si][:], vp[:])

                        # -- attention for chunk j --
                        tmax = min(4 * j + 3, NT - 1)
                        aop = [psA.tile([P, SC], F32, tag="ao", name=f"ao{h}")
                               for h in range(HPC)]
                        zacc = [zpool.tile([P, SC], F32, tag="z",
                                           name=f"z{h}")
                                for h in range(HPC)]
                        items = [(t, h) for t in range(tmax + 1)
                                 for h in range(HPC)]
                        pend = []

                        def drain_one():
                            t_, h_, at_, o_, w_ = pend.pop(0)
                            mm(aop[h_][:, o_:SC],
                               Vsb[t_][:, h_ * D:(h_ + 1) * D],
                               at_[:, :w_],
                               start=(t_ == 0), stop=(t_ == tmax))

                        for t, h in items:
                            o = max(0, t * P - j * SC)
                            w = SC - o
                            stp = psS.tile([P, SC], F32, tag="st", name="st")
                            mm(stp[:, :w], KT[h][:, t * P:(t + 1) * P],
                               QT[h][:, j * SC + o:(j + 1) * SC],
                               start=True, stop=True)
                            at = atp.tile([P, SC], BF16, tag="at", name="at")
                            nc.scalar.activation(at[:, :w], stp[:, :w], EXP,
                                                 scale=inv_sqrt_d)
                            wt_sb = wtp.tile([P, SC], BF16, tag="wt",
                                             name="wt")
                            nc.sync.dma_start(
                                wt_sb[:, :w],
                                wt[h, t * P:(t + 1) * P,
                                   j * SC + o:(j + 1) * SC])
                            nc.vector.tensor_mul(at[:, :w], at[:, :w],
                                                 wt_sb[:, :w])
                            # Z accumulation: h=0 chain on DVE, h=1 on GPSIMD
                            eng = nc.vector if h == 0 else nc.gpsimd
                            if t == 0:
                                eng.tensor_copy(zacc[h][:], at[:])
                            else:
                                eng.tensor_tensor(zacc[h][:, o:SC],
                                                  zacc[h][:, o:SC],
                                                  at[:, :w], ADD)
                            pend.append((t, h, at, o, w))
                            if len(pend) >= c["lag"]:
                                drain_one()
                        while pend:
                            drain_one()

                        for h in range(HPC):
                            zb = zbp.tile([P, SC], F32, tag="zb", name="zb")
                            nc.gpsimd.partition_all_reduce(
                                zb[:], zacc[h][:], channels=P, reduce_op=RADD)
                            rb = rbp.tile([P, SC], F32R, tag="rb", name="rb")
                            nc.vector.reciprocal(rb[:], zb[:])
                            nc.vector.tensor_tensor(
                                AOT[h][:, j * SC:(j + 1) * SC], aop[h][:],
                                rb[:], MULT)

                        # -- output projection for s-tiles of chunk j --
                        for m in range(4 * j, 4 * j + 4):
                            ysb = ypool.tile([P, HID], BF16, tag="y",
                                             name="ysb")
                            for n in range(NJ):
                                yps = psY.tile([P, SC], F32, tag="yy",
                                               name="ps_y")
                                for h in range(HPC):
                                    mm(yps[:], AOT[h][:, m * P:(m + 1) * P],
                                       wo_sb[h][:, n * SC:(n + 1) * SC],
                                       start=(h == 0), stop=(h == HPC - 1))
                                nc.scalar.copy(
                                    ysb[:, n * SC:(n + 1) * SC], yps[:])
                            nc.sync.dma_start(y[m * P:(m + 1) * P, :],
                                              ysb[:])

    nc.compile()
    return nc


def _get_nc():
    if "nc" not in _CACHE:
        _CACHE["nc"] = _build_nc()
    return _CACHE["nc"]


def make_in_maps(hidden_states, idx, valid, geo_bias, Wq, Wk, Wv, Wo):
    """Host-side sharding/layout prep: returns the 8 per-core input maps."""
    bf16 = _np_bf16()
    hs = np.ascontiguousarray(np.asarray(hidden_states, np.float32)[0])
    idx = np.asarray(idx).astype(np.int64)
    valid = np.asarray(valid).astype(bool)

    # hsT [HID, S] -> [P, NK, S]  ((p, k, s) = hsT[k*P+p, s])
    hsT = np.ascontiguousarray(hs.T)
    hsr = np.ascontiguousarray(
        hsT.reshape(NK, P, S).transpose(1, 0, 2)).astype(bf16)

    srange = np.arange(S)
    cmask = ((idx <= srange[:, None]) & valid).ravel()
    flat = (idx * S + srange[:, None]).ravel()[cmask]
    eg = np.exp(np.asarray(geo_bias, np.float64))          # [H, S, K]

    def shard_w(Wfull, sl):
        # W[sl].T [HID, CPC] -> [P, NK, CPC]
        wT = np.ascontiguousarray(np.asarray(Wfull)[sl].T)
        return np.ascontiguousarray(
            wT.reshape(NK, P, CPC).transpose(1, 0, 2)).astype(bf16)

    in_maps = []
    for cix in range(NCORES):
        h0 = HPC * cix
        sl = slice(h0 * D, (h0 + HPC) * D)
        wt_c = np.empty((HPC, S, S), bf16)
        for hh in range(HPC):
            wt_c[hh] = (np.bincount(flat,
                                    weights=eg[h0 + hh].ravel()[cmask],
                                    minlength=S * S)
                        .reshape(S, S).astype(bf16))
        in_maps.append({
            "hsr": hsr,
            "wqr": shard_w(Wq, sl),
            "wkr": shard_w(Wk, sl),
            "wvr": shard_w(Wv, sl),
            "wor": np.ascontiguousarray(
                np.asarray(Wo)[:, sl].T).astype(bf16),
            "wt": wt_c,
        })
    return in_maps


def kernel(hidden_states, idx, valid, geo_bias, Wq, Wk, Wv, Wo, bo):
    from concourse import bass_utils

    nc = _get_nc()
    in_maps = make_in_maps(hidden_states, idx, valid, geo_bias, Wq, Wk, Wv,
                           Wo)
    res = bass_utils.run_bass_kernel_spmd(nc, in_maps,
                                          core_ids=list(range(NCORES)))
    out = np.zeros((S, HID), np.float32)
    for r in res.results:
        out += r["y"].astype(np.float32)
    out += np.asarray(bo, np.float32)
    return out.reshape(B, S, HID)
